# revision 18
# baseline (speedup 1.0000x reference)
"""AdaLayerNormZeroSingle on 8 TRN2 NeuronCores (Bass/Tile), bf16-staged.

reference:
    e = silu(emb) @ W.T + b                  # [1, 9216]
    shift, scale, gate = e.reshape(3072, 3).unbind(-1)
    xn = layernorm(x, eps=1e-6)              # no affine
    out = xn * scale + shift ; return (out, gate)

Sharding: x row-sharded (1024 seq rows/core); W row-sharded (1152 out
features/core), host-permuted so the AllGather yields [shift|scale|gate]
contiguously and host-transposed so the contraction dim lands on SBUF
partitions. b is folded into the GEMV as a 25th contraction chunk whose
activation input is silu^-1(1). All large tensors staged as bf16.
"""

import os
import sys

if "/opt/trn_rl_repo" not in sys.path:
    sys.path.insert(0, "/opt/trn_rl_repo")

import numpy as np

N_CORES = 8
SEQ = 8192
DIM = 3072
JDIM = 3 * DIM            # 9216
SEQ_SH = SEQ // N_CORES   # 1024 rows per core
J_SH = JDIM // N_CORES    # 1152 out-features per core
P = 128
KC = DIM // P             # 24 real contraction chunks
KCB = KC + 1              # +1 chunk folding the bias
WT_ROWS = KCB * P         # 3200
ROW_TILES = SEQ_SH // (2 * P)  # 4 big tiles of [128, 2, 3072]
EPS = 1e-6
SILU_INV_1 = 1.278464542761074  # silu(SILU_INV_1) == 1

_CACHE = {}


def _build():
    import concourse.bass as bass
    import concourse.tile as tile
    from concourse import bacc, mybir

    f32 = mybir.dt.float32
    bf16 = mybir.dt.bfloat16

    nc = bacc.Bacc("TRN2", target_bir_lowering=False, debug=False,
                   num_devices=N_CORES)

    x_ext = nc.declare_dram_parameter("x", [SEQ_SH, DIM], bf16, isOutput=False)
    emb_ext = nc.declare_dram_parameter("emb", [P, KCB], f32, isOutput=False)
    wt_ext = nc.declare_dram_parameter("wt", [WT_ROWS, J_SH], bf16, isOutput=False)
    y_ext = nc.declare_dram_parameter("y", [SEQ_SH, DIM], bf16, isOutput=True)
    gate_ext = nc.declare_dram_parameter("gate", [1, DIM], bf16, isOutput=True)

    AF = mybir.ActivationFunctionType
    OP = mybir.AluOpType

    WTC = 5                      # k-chunks per wt DMA tile
    N_WT = KCB // WTC            # 5 wt tiles

    with tile.TileContext(nc) as tc:
        with (
            tc.tile_pool(name="const", bufs=1) as const,
            tc.tile_pool(name="wtp", bufs=5) as wtp,
            tc.tile_pool(name="xin", bufs=3) as xin,
            tc.tile_pool(name="xnp", bufs=3) as xnp,
            tc.tile_pool(name="outp", bufs=3) as outp,
            tc.tile_pool(name="stat", bufs=4) as stat,
            tc.tile_pool(name="psum", bufs=1, space="PSUM") as psum,
            tc.tile_pool(name="dram", bufs=1, space="DRAM") as dram,
        ):
            # ---------------- GEMV: e = [Wp_shard|b] @ [silu(emb);1] ------
            emb_t = const.tile([P, KCB], f32)
            nc.sync.dma_start(emb_t[:, :], emb_ext[:, :])
            s_t = const.tile([P, KCB], bf16)
            nc.scalar.activation(s_t[:, :], emb_t[:, :], AF.Silu)

            pe = psum.tile([1, 3, 512], f32, tag="ge")
            NB = ((0, 512), (512, 512), (1024, 128))  # 1152 = 512+512+128
            last_wt_dma = None
            for c in range(KCB):
                wt_c = wtp.tile([P, J_SH], bf16, name=f"wt_{c}", tag="wt",
                                bufs=KCB)
                last_wt_dma = nc.sync.dma_start(
                    wt_c[:, :], wt_ext[c * P:(c + 1) * P, :])
                for nb, (n0, nsz) in enumerate(NB):
                    nc.tensor.matmul(
                        pe[0:1, nb, 0:nsz],
                        s_t[:, c:c + 1],
                        wt_c[:, n0:n0 + nsz],
                        start=(c == 0),
                        stop=(c == KCB - 1),
                    )

            e_bf = const.tile([1, J_SH], bf16)
            nc.scalar.copy(e_bf[0:1, :],
                           pe[0:1, :, :].rearrange("p a f -> p (a f)")[0:1, 0:J_SH])

            # ---------------- AllGather e (18 KB, bf16) -------------------
            eb_in = dram.tile([1, J_SH], bf16)
            eb_out = dram.tile([1, JDIM], bf16)
            nc.gpsimd.dma_start(eb_in[:, :], e_bf[:, :])
            nc.gpsimd.collective_compute(
                "AllGather",
                OP.bypass,
                replica_groups=[list(range(N_CORES))],
                ins=[eb_in[0:1, :].opt()],
                outs=[eb_out[0:1, :].opt()],
            )
            # shift/scale broadcast to 128 partitions via stride-0 DMA
            shift_b = const.tile([P, DIM], bf16)
            scale_b = const.tile([P, DIM], bf16)
            H = DIM // 2
            nc.gpsimd.dma_start(
                scale_b[:, 0:H],
                eb_out[0:1, DIM:DIM + H].partition_broadcast(P))
            nc.sync.dma_start(
                scale_b[:, H:DIM],
                eb_out[0:1, DIM + H:2 * DIM].partition_broadcast(P))
            nc.gpsimd.dma_start(
                shift_b[:, 0:H],
                eb_out[0:1, 0:H].partition_broadcast(P))
            nc.sync.dma_start(
                shift_b[:, H:DIM],
                eb_out[0:1, H:DIM].partition_broadcast(P))
            # gate = E[2*DIM:3*DIM] straight to output
            nc.gpsimd.dma_start(gate_ext[0:1, :], eb_out[0:1, 2 * DIM:3 * DIM])

            # ---------------- LayerNorm: stats + xn for all tiles ----------
            eps_t = const.tile([P, 1], f32)
            nc.vector.memset(eps_t[:, :], EPS)

            xns = []
            last_xn = None
            for t in range(ROW_TILES):
                xn = xnp.tile([P, 2, DIM], bf16, name=f"xn_{t}", tag="xn",
                              bufs=ROW_TILES)
                xns.append(xn)
                for a in range(2):
                    r0 = (2 * t + a) * P
                    xt = xin.tile([P, DIM], bf16, name=f"xt_{t}{a}", tag="xt",
                                  bufs=8)
                    xeng = nc.gpsimd if a == 0 else nc.sync
                    xdma = xeng.dma_start(xt[:, :], x_ext[r0:r0 + P, :])
                    st6 = stat.tile([P, 36], f32, name=f"st6_{t}{a}", tag="st6")
                    for g in range(DIM // 512):
                        nc.vector.bn_stats(
                            st6[:, g * 6:(g + 1) * 6],
                            xt[:, g * 512:(g + 1) * 512],
                        )
                    mv = stat.tile([P, 2], f32, name=f"mv_{t}{a}", tag="mv")
                    nc.vector.bn_aggr(mv[:, :], st6[:, :])

                    sd = stat.tile([P, 1], f32, name=f"sd_{t}{a}", tag="sd")
                    nc.scalar.activation(sd[:, :], mv[:, 1:2], AF.Sqrt,
                                         bias=eps_t[:, 0:1])
                    istd = stat.tile([P, 1], f32, name=f"istd_{t}{a}", tag="istd")
                    nc.vector.reciprocal(istd[:, :], sd[:, :])
                    nmi = stat.tile([P, 1], f32, name=f"nmi_{t}{a}", tag="nmi")
                    last_vec = nc.vector.tensor_scalar(nmi[:, :], mv[:, 0:1],
                                            istd[:, 0:1], -1.0,
                                            OP.mult, OP.mult)

                    last_xn = nc.scalar.activation(
                        xn[:, a, :], xt[:, :], AF.Identity,
                        bias=nmi[:, 0:1], scale=istd[:, 0:1])

            # ---------------- modulate + store (after the gather) ----------
            import concourse.bass as _b
            def _bc2(ap2d, reps):
                # [128, DIM] -> [128, reps(0-stride), DIM]
                return _b.AP(tensor=ap2d.tensor, offset=ap2d.offset,
                             ap=[ap2d.ap[0], [0, reps], [1, DIM]])

            scale_v = _bc2(scale_b[:, :], 2)
            shift_v = _bc2(shift_b[:, :], 2)
            for t in range(ROW_TILES):
                r0 = t * 2 * P
                xn = xns[t]
                ot = outp.tile([P, 2, DIM], bf16, name=f"ot_{t}", tag="ot")
                m = nc.vector.tensor_mul(xn[:, :, :], xn[:, :, :], scale_v)
                bass._add_dep_helper(m.ins, last_xn.ins, sync=True,
                                     reason="applies after all xn")
                bass._add_dep_helper(m.ins, last_vec.ins, sync=True,
                                     reason="applies after all stats")
                nc.vector.tensor_add(ot[:, :, :], xn[:, :, :], shift_v)
                for a in range(2):
                    yeng = (nc.scalar, nc.scalar, nc.scalar, nc.scalar,
                            nc.scalar, nc.scalar, nc.sync, nc.gpsimd)[2 * t + a]
                    yeng.dma_start(
                        y_ext[r0 + a * P:r0 + (a + 1) * P, :],
                        ot[:, a, :],
                    )

    nc.compile()
    return nc


def _get_nc():
    if "nc" not in _CACHE:
        _CACHE["nc"] = _build()
    return _CACHE["nc"]


def _shard_inputs(x, emb, W, b):
    import ml_dtypes

    bf = ml_dtypes.bfloat16
    x2 = np.ascontiguousarray(x.reshape(SEQ, DIM)).astype(bf)
    # permute rows of W so the gathered e is [shift | scale | gate]
    p = np.arange(JDIM)
    perm = 3 * (p % DIM) + p // DIM
    Wp = np.asarray(W)[perm]
    bp = np.asarray(b)[perm]
    # emb k-major layout [128, 25]: col c<24 -> emb[c*128+p]; col 24 = silu^-1(1) one-hot
    emb_l = np.zeros((P, KCB), dtype=np.float32)
    emb_l[:, :KC] = np.asarray(emb, dtype=np.float32).reshape(KC, P).T
    emb_l[0, KC] = SILU_INV_1
    in_maps = []
    for i in range(N_CORES):
        wt = np.zeros((WT_ROWS, J_SH), dtype=bf)
        wt[:DIM] = Wp[i * J_SH:(i + 1) * J_SH].T.astype(bf)
        wt[DIM] = bp[i * J_SH:(i + 1) * J_SH].astype(bf)
        in_maps.append({
            "x": x2[i * SEQ_SH:(i + 1) * SEQ_SH],
            "emb": emb_l,
            "wt": wt,
        })
    return in_maps


def kernel(x, emb, W, b):
    from concourse.bass_utils import run_bass_kernel_spmd

    nc = _get_nc()
    in_maps = _shard_inputs(x, emb, W, b)
    trace = bool(os.environ.get("KERNEL_TRACE"))
    if trace:
        try:
            import trace_shim
            trace_shim.install()
        except Exception:
            trace = False
    res = run_bass_kernel_spmd(nc, in_maps, core_ids=list(range(N_CORES)),
                               trace=trace)
    _CACHE["last_result"] = res

    out = np.empty((1, SEQ, DIM), dtype=np.float32)
    for i in range(N_CORES):
        out[0, i * SEQ_SH:(i + 1) * SEQ_SH, :] = res.results[i]["y"].astype(
            np.float32)
    gate = res.results[0]["gate"].reshape(1, DIM).astype(np.float32)
    return out, gate


# revision 19
# speedup vs baseline: 1.1115x; 1.1115x over previous
"""AdaLayerNormZeroSingle on 8 TRN2 NeuronCores (Bass/Tile), bf16-staged.

reference:
    e = silu(emb) @ W.T + b                  # [1, 9216]
    shift, scale, gate = e.reshape(3072, 3).unbind(-1)
    xn = layernorm(x, eps=1e-6)              # no affine
    out = xn * scale + shift ; return (out, gate)

Sharding: x row-sharded (1024 seq rows/core); W row-sharded (1152 out
features/core), host-permuted so the AllGather yields [shift|scale|gate]
contiguously and host-transposed so the contraction dim lands on SBUF
partitions. b is folded into the GEMV as a 25th contraction chunk whose
activation input is silu^-1(1). All large tensors staged as bf16.
"""

import os
import sys

if "/opt/trn_rl_repo" not in sys.path:
    sys.path.insert(0, "/opt/trn_rl_repo")

import numpy as np

N_CORES = 8
SEQ = 8192
DIM = 3072
JDIM = 3 * DIM            # 9216
SEQ_SH = SEQ // N_CORES   # 1024 rows per core
J_SH = JDIM // N_CORES    # 1152 out-features per core
P = 128
KC = DIM // P             # 24 real contraction chunks
KCB = KC + 1              # +1 chunk folding the bias
WT_ROWS = KCB * P         # 3200
ROW_TILES = SEQ_SH // (2 * P)  # 4 big tiles of [128, 2, 3072]
EPS = 1e-6
SILU_INV_1 = 1.278464542761074  # silu(SILU_INV_1) == 1

_CACHE = {}


def _build():
    import concourse.bass as bass
    import concourse.tile as tile
    from concourse import bacc, mybir

    f32 = mybir.dt.float32
    bf16 = mybir.dt.bfloat16

    nc = bacc.Bacc("TRN2", target_bir_lowering=False, debug=False,
                   num_devices=N_CORES)

    x_ext = nc.declare_dram_parameter("x", [SEQ_SH, DIM], bf16, isOutput=False)
    emb_ext = nc.declare_dram_parameter("emb", [P, KCB], f32, isOutput=False)
    wt_ext = nc.declare_dram_parameter("wt", [WT_ROWS, J_SH], bf16, isOutput=False)
    y_ext = nc.declare_dram_parameter("y", [SEQ_SH, DIM], bf16, isOutput=True)
    gate_ext = nc.declare_dram_parameter("gate", [1, DIM], bf16, isOutput=True)

    AF = mybir.ActivationFunctionType
    OP = mybir.AluOpType

    WTC = 5                      # k-chunks per wt DMA tile
    N_WT = KCB // WTC            # 5 wt tiles

    with tile.TileContext(nc) as tc:
        with (
            tc.tile_pool(name="const", bufs=1) as const,
            tc.tile_pool(name="wtp", bufs=5) as wtp,
            tc.tile_pool(name="xin", bufs=3) as xin,
            tc.tile_pool(name="xnp", bufs=3) as xnp,
            tc.tile_pool(name="outp", bufs=3) as outp,
            tc.tile_pool(name="stat", bufs=4) as stat,
            tc.tile_pool(name="psum", bufs=1, space="PSUM") as psum,
            tc.tile_pool(name="dram", bufs=1, space="DRAM") as dram,
        ):
            # ---------------- GEMV: e = [Wp_shard|b] @ [silu(emb);1] ------
            emb_t = const.tile([P, KCB], f32)
            nc.sync.dma_start(emb_t[:, :], emb_ext[:, :])
            s_t = const.tile([P, KCB], bf16)
            nc.scalar.activation(s_t[:, :], emb_t[:, :], AF.Silu)

            pe = psum.tile([1, 3, 512], f32, tag="ge")
            NB = ((0, 512), (512, 512), (1024, 128))  # 1152 = 512+512+128
            last_wt_dma = None
            for c in range(KCB):
                wt_c = wtp.tile([P, J_SH], bf16, name=f"wt_{c}", tag="wt",
                                bufs=KCB)
                last_wt_dma = nc.sync.dma_start(
                    wt_c[:, :], wt_ext[c * P:(c + 1) * P, :])
                for nb, (n0, nsz) in enumerate(NB):
                    nc.tensor.matmul(
                        pe[0:1, nb, 0:nsz],
                        s_t[:, c:c + 1],
                        wt_c[:, n0:n0 + nsz],
                        start=(c == 0),
                        stop=(c == KCB - 1),
                    )

            e_bf = const.tile([1, J_SH], bf16)
            nc.scalar.copy(e_bf[0:1, :],
                           pe[0:1, :, :].rearrange("p a f -> p (a f)")[0:1, 0:J_SH])

            # ---------------- AllGather e (18 KB, bf16) -------------------
            eb_in = dram.tile([1, J_SH], bf16)
            eb_out = dram.tile([1, JDIM], bf16)
            nc.gpsimd.dma_start(eb_in[:, :], e_bf[:, :])
            nc.gpsimd.collective_compute(
                "AllGather",
                OP.bypass,
                replica_groups=[list(range(N_CORES))],
                ins=[eb_in[0:1, :].opt()],
                outs=[eb_out[0:1, :].opt()],
            )
            # shift/scale broadcast to 128 partitions via stride-0 DMA
            shift_b = const.tile([P, DIM], bf16)
            scale_b = const.tile([P, DIM], bf16)
            nc.gpsimd.dma_start(scale_b[:, :],
                                eb_out[0:1, DIM:2 * DIM].partition_broadcast(P))
            nc.gpsimd.dma_start(shift_b[:, :],
                                eb_out[0:1, 0:DIM].partition_broadcast(P))
            # gate = E[2*DIM:3*DIM] straight to output
            nc.gpsimd.dma_start(gate_ext[0:1, :], eb_out[0:1, 2 * DIM:3 * DIM])

            # ---------------- LayerNorm: stats + xn for all tiles ----------
            eps_t = const.tile([P, 1], f32)
            nc.vector.memset(eps_t[:, :], EPS)

            xns = []
            last_xn = None
            for t in range(ROW_TILES):
                xn = xnp.tile([P, 2, DIM], bf16, name=f"xn_{t}", tag="xn",
                              bufs=ROW_TILES)
                xns.append(xn)
                for a in range(2):
                    r0 = (2 * t + a) * P
                    xt = xin.tile([P, DIM], bf16, name=f"xt_{t}{a}", tag="xt",
                                  bufs=8)
                    xeng = nc.gpsimd if a == 0 else nc.sync
                    xdma = xeng.dma_start(xt[:, :], x_ext[r0:r0 + P, :])
                    st6 = stat.tile([P, 36], f32, name=f"st6_{t}{a}", tag="st6")
                    for g in range(DIM // 512):
                        nc.vector.bn_stats(
                            st6[:, g * 6:(g + 1) * 6],
                            xt[:, g * 512:(g + 1) * 512],
                        )
                    mv = stat.tile([P, 2], f32, name=f"mv_{t}{a}", tag="mv")
                    nc.vector.bn_aggr(mv[:, :], st6[:, :])

                    sd = stat.tile([P, 1], f32, name=f"sd_{t}{a}", tag="sd")
                    nc.scalar.activation(sd[:, :], mv[:, 1:2], AF.Sqrt,
                                         bias=eps_t[:, 0:1])
                    istd = stat.tile([P, 1], f32, name=f"istd_{t}{a}", tag="istd")
                    nc.vector.reciprocal(istd[:, :], sd[:, :])
                    nmi = stat.tile([P, 1], f32, name=f"nmi_{t}{a}", tag="nmi")
                    last_vec = nc.vector.tensor_scalar(nmi[:, :], mv[:, 0:1],
                                            istd[:, 0:1], -1.0,
                                            OP.mult, OP.mult)

                    last_xn = nc.scalar.activation(
                        xn[:, a, :], xt[:, :], AF.Identity,
                        bias=nmi[:, 0:1], scale=istd[:, 0:1])

            # ---------------- modulate + store (after the gather) ----------
            import concourse.bass as _b
            def _bc2(ap2d, reps):
                # [128, DIM] -> [128, reps(0-stride), DIM]
                return _b.AP(tensor=ap2d.tensor, offset=ap2d.offset,
                             ap=[ap2d.ap[0], [0, reps], [1, DIM]])

            scale_v = _bc2(scale_b[:, :], 2)
            shift_v = _bc2(shift_b[:, :], 2)
            for t in range(ROW_TILES):
                r0 = t * 2 * P
                xn = xns[t]
                ot = outp.tile([P, 2, DIM], bf16, name=f"ot_{t}", tag="ot")
                m = nc.vector.tensor_mul(xn[:, :, :], xn[:, :, :], scale_v)
                bass._add_dep_helper(m.ins, last_xn.ins, sync=True,
                                     reason="applies after all xn")
                bass._add_dep_helper(m.ins, last_vec.ins, sync=True,
                                     reason="applies after all stats")
                nc.vector.tensor_add(ot[:, :, :], xn[:, :, :], shift_v)
                for a in range(2):
                    nc.scalar.dma_start(
                        y_ext[r0 + a * P:r0 + (a + 1) * P, :],
                        ot[:, a, :],
                    )

    nc.compile()
    return nc


def _get_nc():
    if "nc" not in _CACHE:
        _CACHE["nc"] = _build()
    return _CACHE["nc"]


def _shard_inputs(x, emb, W, b):
    import ml_dtypes

    bf = ml_dtypes.bfloat16
    x2 = np.ascontiguousarray(x.reshape(SEQ, DIM)).astype(bf)
    # permute rows of W so the gathered e is [shift | scale | gate]
    p = np.arange(JDIM)
    perm = 3 * (p % DIM) + p // DIM
    Wp = np.asarray(W)[perm]
    bp = np.asarray(b)[perm]
    # emb k-major layout [128, 25]: col c<24 -> emb[c*128+p]; col 24 = silu^-1(1) one-hot
    emb_l = np.zeros((P, KCB), dtype=np.float32)
    emb_l[:, :KC] = np.asarray(emb, dtype=np.float32).reshape(KC, P).T
    emb_l[0, KC] = SILU_INV_1
    in_maps = []
    for i in range(N_CORES):
        wt = np.zeros((WT_ROWS, J_SH), dtype=bf)
        wt[:DIM] = Wp[i * J_SH:(i + 1) * J_SH].T.astype(bf)
        wt[DIM] = bp[i * J_SH:(i + 1) * J_SH].astype(bf)
        in_maps.append({
            "x": x2[i * SEQ_SH:(i + 1) * SEQ_SH],
            "emb": emb_l,
            "wt": wt,
        })
    return in_maps


def kernel(x, emb, W, b):
    from concourse.bass_utils import run_bass_kernel_spmd

    nc = _get_nc()
    in_maps = _shard_inputs(x, emb, W, b)
    trace = bool(os.environ.get("KERNEL_TRACE"))
    if trace:
        try:
            import trace_shim
            trace_shim.install()
        except Exception:
            trace = False
    res = run_bass_kernel_spmd(nc, in_maps, core_ids=list(range(N_CORES)),
                               trace=trace)
    _CACHE["last_result"] = res

    out = np.empty((1, SEQ, DIM), dtype=np.float32)
    for i in range(N_CORES):
        out[0, i * SEQ_SH:(i + 1) * SEQ_SH, :] = res.results[i]["y"].astype(
            np.float32)
    gate = res.results[0]["gate"].reshape(1, DIM).astype(np.float32)
    return out, gate


# revision 20
# speedup vs baseline: 1.1942x; 1.0744x over previous
"""AdaLayerNormZeroSingle on 8 TRN2 NeuronCores (Bass/Tile), bf16-staged.

reference:
    e = silu(emb) @ W.T + b                  # [1, 9216]
    shift, scale, gate = e.reshape(3072, 3).unbind(-1)
    xn = layernorm(x, eps=1e-6)              # no affine
    out = xn * scale + shift ; return (out, gate)

Sharding: x row-sharded (1024 seq rows/core); W row-sharded (1152 out
features/core), host-permuted so the AllGather yields [shift|scale|gate]
contiguously and host-transposed so the contraction dim lands on SBUF
partitions. b is folded into the GEMV as a 25th contraction chunk whose
activation input is silu^-1(1). All large tensors staged as bf16.
"""

import os
import sys

if "/opt/trn_rl_repo" not in sys.path:
    sys.path.insert(0, "/opt/trn_rl_repo")

import numpy as np

N_CORES = 8
SEQ = 8192
DIM = 3072
JDIM = 3 * DIM            # 9216
SEQ_SH = SEQ // N_CORES   # 1024 rows per core
J_SH = JDIM // N_CORES    # 1152 out-features per core
P = 128
KC = DIM // P             # 24 real contraction chunks
KCB = KC + 1              # +1 chunk folding the bias
WT_ROWS = KCB * P         # 3200
ROW_TILES = SEQ_SH // (2 * P)  # 4 big tiles of [128, 2, 3072]
EPS = 1e-6
SILU_INV_1 = 1.278464542761074  # silu(SILU_INV_1) == 1

_CACHE = {}


def _build():
    import concourse.bass as bass
    import concourse.tile as tile
    from concourse import bacc, mybir

    f32 = mybir.dt.float32
    bf16 = mybir.dt.bfloat16

    nc = bacc.Bacc("TRN2", target_bir_lowering=False, debug=False,
                   num_devices=N_CORES)

    x_ext = nc.declare_dram_parameter("x", [SEQ_SH, DIM], bf16, isOutput=False)
    emb_ext = nc.declare_dram_parameter("emb", [P, KCB], f32, isOutput=False)
    wt_ext = nc.declare_dram_parameter("wt", [WT_ROWS, J_SH], bf16, isOutput=False)
    y_ext = nc.declare_dram_parameter("y", [SEQ_SH, DIM], bf16, isOutput=True)
    gate_ext = nc.declare_dram_parameter("gate", [1, DIM], bf16, isOutput=True)

    AF = mybir.ActivationFunctionType
    OP = mybir.AluOpType

    WTC = 5                      # k-chunks per wt DMA tile
    N_WT = KCB // WTC            # 5 wt tiles

    with tile.TileContext(nc) as tc:
        with (
            tc.tile_pool(name="const", bufs=1) as const,
            tc.tile_pool(name="wtp", bufs=5) as wtp,
            tc.tile_pool(name="xin", bufs=3) as xin,
            tc.tile_pool(name="xnp", bufs=3) as xnp,
            tc.tile_pool(name="outp", bufs=3) as outp,
            tc.tile_pool(name="stat", bufs=4) as stat,
            tc.tile_pool(name="psum", bufs=1, space="PSUM") as psum,
            tc.tile_pool(name="dram", bufs=1, space="DRAM") as dram,
        ):
            # ---------------- GEMV: e = [Wp_shard|b] @ [silu(emb);1] ------
            emb_t = const.tile([P, KCB], f32)
            nc.sync.dma_start(emb_t[:, :], emb_ext[:, :])
            s_t = const.tile([P, KCB], bf16)
            nc.scalar.activation(s_t[:, :], emb_t[:, :], AF.Silu)

            pe = psum.tile([1, 3, 512], f32, tag="ge")
            NB = ((0, 512), (512, 512), (1024, 128))  # 1152 = 512+512+128
            last_wt_dma = None
            for c in range(KCB):
                wt_c = wtp.tile([P, J_SH], bf16, name=f"wt_{c}", tag="wt",
                                bufs=KCB)
                weng = nc.sync if c % 2 == 0 else nc.gpsimd
                last_wt_dma = weng.dma_start(
                    wt_c[:, :], wt_ext[c * P:(c + 1) * P, :])
                for nb, (n0, nsz) in enumerate(NB):
                    nc.tensor.matmul(
                        pe[0:1, nb, 0:nsz],
                        s_t[:, c:c + 1],
                        wt_c[:, n0:n0 + nsz],
                        start=(c == 0),
                        stop=(c == KCB - 1),
                    )

            e_bf = const.tile([1, J_SH], bf16)
            nc.scalar.copy(e_bf[0:1, :],
                           pe[0:1, :, :].rearrange("p a f -> p (a f)")[0:1, 0:J_SH])

            # ---------------- AllGather e (18 KB, bf16) -------------------
            eb_in = dram.tile([1, J_SH], bf16)
            eb_out = dram.tile([1, JDIM], bf16)
            nc.gpsimd.dma_start(eb_in[:, :], e_bf[:, :])
            nc.gpsimd.collective_compute(
                "AllGather",
                OP.bypass,
                replica_groups=[list(range(N_CORES))],
                ins=[eb_in[0:1, :].opt()],
                outs=[eb_out[0:1, :].opt()],
            )
            # shift/scale broadcast to 128 partitions via stride-0 DMA
            shift_b = const.tile([P, DIM], bf16)
            scale_b = const.tile([P, DIM], bf16)
            nc.gpsimd.dma_start(scale_b[:, :],
                                eb_out[0:1, DIM:2 * DIM].partition_broadcast(P))
            nc.gpsimd.dma_start(shift_b[:, :],
                                eb_out[0:1, 0:DIM].partition_broadcast(P))
            # gate = E[2*DIM:3*DIM] straight to output
            nc.gpsimd.dma_start(gate_ext[0:1, :], eb_out[0:1, 2 * DIM:3 * DIM])

            # ---------------- LayerNorm: stats + xn for all tiles ----------
            eps_t = const.tile([P, 1], f32)
            nc.vector.memset(eps_t[:, :], EPS)

            xns = []
            last_xn = None
            for t in range(ROW_TILES):
                xn = xnp.tile([P, 2, DIM], bf16, name=f"xn_{t}", tag="xn",
                              bufs=ROW_TILES)
                xns.append(xn)
                for a in range(2):
                    r0 = (2 * t + a) * P
                    xt = xin.tile([P, DIM], bf16, name=f"xt_{t}{a}", tag="xt",
                                  bufs=8)
                    xdma = nc.sync.dma_start(xt[:, :], x_ext[r0:r0 + P, :])
                    st6 = stat.tile([P, 36], f32, name=f"st6_{t}{a}", tag="st6")
                    for g in range(DIM // 512):
                        nc.vector.bn_stats(
                            st6[:, g * 6:(g + 1) * 6],
                            xt[:, g * 512:(g + 1) * 512],
                        )
                    mv = stat.tile([P, 2], f32, name=f"mv_{t}{a}", tag="mv")
                    nc.vector.bn_aggr(mv[:, :], st6[:, :])

                    sd = stat.tile([P, 1], f32, name=f"sd_{t}{a}", tag="sd")
                    nc.scalar.activation(sd[:, :], mv[:, 1:2], AF.Sqrt,
                                         bias=eps_t[:, 0:1])
                    istd = stat.tile([P, 1], f32, name=f"istd_{t}{a}", tag="istd")
                    nc.vector.reciprocal(istd[:, :], sd[:, :])
                    nmi = stat.tile([P, 1], f32, name=f"nmi_{t}{a}", tag="nmi")
                    last_vec = nc.vector.tensor_scalar(nmi[:, :], mv[:, 0:1],
                                            istd[:, 0:1], -1.0,
                                            OP.mult, OP.mult)

                    last_xn = nc.scalar.activation(
                        xn[:, a, :], xt[:, :], AF.Identity,
                        bias=nmi[:, 0:1], scale=istd[:, 0:1])

            # ---------------- modulate + store (after the gather) ----------
            import concourse.bass as _b
            def _bc2(ap2d, reps):
                # [128, DIM] -> [128, reps(0-stride), DIM]
                return _b.AP(tensor=ap2d.tensor, offset=ap2d.offset,
                             ap=[ap2d.ap[0], [0, reps], [1, DIM]])

            scale_v = _bc2(scale_b[:, :], 2)
            shift_v = _bc2(shift_b[:, :], 2)
            for t in range(ROW_TILES):
                r0 = t * 2 * P
                xn = xns[t]
                ot = outp.tile([P, 2, DIM], bf16, name=f"ot_{t}", tag="ot")
                m = nc.vector.tensor_mul(xn[:, :, :], xn[:, :, :], scale_v)
                bass._add_dep_helper(m.ins, last_xn.ins, sync=True,
                                     reason="applies after all xn")
                bass._add_dep_helper(m.ins, last_vec.ins, sync=True,
                                     reason="applies after all stats")
                nc.vector.tensor_add(ot[:, :, :], xn[:, :, :], shift_v)
                for a in range(2):
                    nc.scalar.dma_start(
                        y_ext[r0 + a * P:r0 + (a + 1) * P, :],
                        ot[:, a, :],
                    )

    nc.compile()
    return nc


def _get_nc():
    if "nc" not in _CACHE:
        _CACHE["nc"] = _build()
    return _CACHE["nc"]


def _shard_inputs(x, emb, W, b):
    import ml_dtypes

    bf = ml_dtypes.bfloat16
    x2 = np.ascontiguousarray(x.reshape(SEQ, DIM)).astype(bf)
    # permute rows of W so the gathered e is [shift | scale | gate]
    p = np.arange(JDIM)
    perm = 3 * (p % DIM) + p // DIM
    Wp = np.asarray(W)[perm]
    bp = np.asarray(b)[perm]
    # emb k-major layout [128, 25]: col c<24 -> emb[c*128+p]; col 24 = silu^-1(1) one-hot
    emb_l = np.zeros((P, KCB), dtype=np.float32)
    emb_l[:, :KC] = np.asarray(emb, dtype=np.float32).reshape(KC, P).T
    emb_l[0, KC] = SILU_INV_1
    in_maps = []
    for i in range(N_CORES):
        wt = np.zeros((WT_ROWS, J_SH), dtype=bf)
        wt[:DIM] = Wp[i * J_SH:(i + 1) * J_SH].T.astype(bf)
        wt[DIM] = bp[i * J_SH:(i + 1) * J_SH].astype(bf)
        in_maps.append({
            "x": x2[i * SEQ_SH:(i + 1) * SEQ_SH],
            "emb": emb_l,
            "wt": wt,
        })
    return in_maps


def kernel(x, emb, W, b):
    from concourse.bass_utils import run_bass_kernel_spmd

    nc = _get_nc()
    in_maps = _shard_inputs(x, emb, W, b)
    trace = bool(os.environ.get("KERNEL_TRACE"))
    if trace:
        try:
            import trace_shim
            trace_shim.install()
        except Exception:
            trace = False
    res = run_bass_kernel_spmd(nc, in_maps, core_ids=list(range(N_CORES)),
                               trace=trace)
    _CACHE["last_result"] = res

    out = np.empty((1, SEQ, DIM), dtype=np.float32)
    for i in range(N_CORES):
        out[0, i * SEQ_SH:(i + 1) * SEQ_SH, :] = res.results[i]["y"].astype(
            np.float32)
    gate = res.results[0]["gate"].reshape(1, DIM).astype(np.float32)
    return out, gate
